# revision 1
# baseline (speedup 1.0000x reference)
"""MoE grouped-FFN kernel for Trainium2 (8 NeuronCores, expert-parallel).

Problem: x [1, 2048, 1024] fp32, 32 experts x 64 tokens each,
per-expert FFN 1024 -> 4096 (gelu) -> 1024.

Sharding: expert-parallel, 4 experts per core. Tokens are statically
pre-chunked per expert (dim 1 == E*C), so each core just gets its 4
experts' token rows + weights; outputs concatenate back. No collectives.

The problem is HBM-bound, so weights/activations stream in bf16 (half of
fp32 traffic), and the first 2 of 32 f-tiles per expert additionally
stream in fp8 e4m3; PSUM accumulation stays fp32. Measured end-to-end
rel err on HW: 1.59e-2 vs the 2e-2 gate with W1@2/32 + W2@3/32 f-tiles
in fp8 (bf16-only: 4.4e-3; symmetric 3/32 measured 1.88e-2 - too close).

Per-core dataflow, software-pipelined over 9 steps per expert (one fp8
mini-chunk + one bf16 remainder + 7 full 512-wide bf16 chunks), with
both weight streams issued from the SP queue in a strictly alternating
w1(g+LEAD), w2(g) order (see _emit_body for why):
  mm1: h[c,nf] += xT[k].T @ W1[k, fchunk]   (tokens on M=64, K-accum in PSUM)
  DVE: hs = bf16(h_psum); PE-transpose hs -> hT [f, c]
  ACT: hT_sb = gelu(hT + b1) (per-partition bias)
  mm2: out[c,512d] += hT[ft].T @ W2[ft, dchunk]  (b2 enters PSUM as a
       rank-1 ones.T @ b2 accumulate)
  DVE evacuates PSUM to bf16; out store rides the Pool/ACT queue.
"""

import os
import numpy as np

E, C, D, F = 32, 64, 1024, 4096
N_CORES = 8
E_LOC = E // N_CORES  # experts per core
P = 128
NMAX = 512  # matmul moving-operand max for 4-byte dtypes
KT1 = D // P  # 8 K-tiles in mm1
FT = F // P  # 32 f-tiles (contraction tiles for mm2)

# Tuning knobs (SBUF budget is ~192KB/partition under Tile).
DEFAULT_CFG = dict(
    w1_chunk=512,  # F columns per W1 DMA (x8 KT1 rows); must equal NMAX
    w2_block=4,    # f-tiles per W2 DMA (x D columns); must equal w1_chunk/128
    w1_bufs=6,
    w2_bufs=8,
    hs_bufs=8,
    ht_bufs=2,
    os_bufs=2,
    ph_bufs=2,
    pt_bufs=2,
    po_bufs=4,     # two experts' worth: next expert's bias matmuls need not
                   # wait for the previous expert's PSUM evacuation
    mm2_lag=2,     # consume chunk g-LAG behind mm1 emission
    w1_lead=3,     # w1 transfer stream leads w2 by this many chunks
    # First fp8_ftiles 128-wide f-tiles of each expert stream W1 columns and
    # W2 rows in e4m3 instead of bf16 (PSUM accum still fp32). Error is
    # sqrt(fraction)-scaled; measured on HW: 1.36e-2 at sym 2/32, 1.59e-2 at
    # W1@2+W2@3 (shipped), 1.88e-2 at sym 3/32, 2.03e-2 at sym 4/32 (FAILS
    # the 2e-2 gate). Do NOT raise further.
    fp8_ftiles=2,
    w1f_bufs=2,
    w2f_bufs=2,
)

_CACHE = {}
LAST_RESULTS = None  # BassKernelResults of the most recent run (for profiling)


def _build_program(act="gelu", repeats=1, cfg=None):
    import contextlib

    import concourse.bacc as bacc
    import concourse.tile as tile
    import concourse.mybir as mybir

    cfg = dict(DEFAULT_CFG, **(cfg or {}))

    f32 = mybir.dt.float32
    bf16 = mybir.dt.bfloat16  # streamed operand dtype; PSUM accum stays fp32
    fp8 = mybir.dt.float8e4
    # CoreSim doesn't implement the Gelu LUTs; "tanh" is a sim-only stand-in
    # used by test.py to validate everything except the activation itself.
    GELU = {
        "gelu": mybir.ActivationFunctionType.Gelu_apprx_tanh,
        "tanh": mybir.ActivationFunctionType.Tanh,
    }[act]
    COPY = mybir.ActivationFunctionType.Copy

    nc = bacc.Bacc("TRN2", target_bir_lowering=False, debug=False)

    q8 = cfg["fp8_ftiles"]  # 128-wide f-tiles of chunk 0 streamed in fp8
    n_ch = F // cfg["w1_chunk"] - 1
    n_fb = FT // cfg["w2_block"] - 1
    assert 0 < q8 < cfg["w2_block"]
    xT_d = nc.declare_dram_parameter("xT", [P, E_LOC, KT1, C], bf16, isOutput=False)
    # w1/w2 arrive host-pre-tiled so every weight DMA is one contiguous read:
    # w1[e, c, p, k, fc] = W1[e, k*128+p, (c+1)*chunk+fc]
    # w2[e, fb, p, j, d] = W2[e, ((fb+1)*block+j)*128+p, d]
    # Chunk 0 is split: its first q8 f-tiles live in w1q/w2q (fp8), the rest
    # in w1h/w2h (bf16).
    w1_d = nc.declare_dram_parameter(
        "w1", [E_LOC, n_ch, P, KT1, cfg["w1_chunk"]], bf16, isOutput=False
    )
    w2_d = nc.declare_dram_parameter(
        "w2", [E_LOC, n_fb, P, cfg["w2_block"], D], bf16, isOutput=False
    )
    w1q_d = nc.declare_dram_parameter(
        "w1q", [E_LOC, P, KT1, q8 * P], fp8, isOutput=False
    )
    w1h_d = nc.declare_dram_parameter(
        "w1h", [E_LOC, P, KT1, cfg["w1_chunk"] - q8 * P], bf16, isOutput=False
    )
    w2q_d = nc.declare_dram_parameter("w2q", [E_LOC, P, q8, D], fp8, isOutput=False)
    w2x_d = nc.declare_dram_parameter("w2x", [E_LOC, P, 1, D], fp8, isOutput=False)
    w2h_d = nc.declare_dram_parameter(
        "w2h", [E_LOC, P, cfg["w2_block"] - q8 - 1, D], bf16, isOutput=False
    )
    b1_d = nc.declare_dram_parameter("b1t", [P, E_LOC, FT], f32, isOutput=False)
    b2_d = nc.declare_dram_parameter("b2s", [1, E_LOC, D], bf16, isOutput=False)
    on_d = nc.declare_dram_parameter("ones1", [1, C], bf16, isOutput=False)
    id_d = nc.declare_dram_parameter("ident", [C, C], bf16, isOutput=False)
    out_d = nc.declare_dram_parameter("out", [E_LOC * C, D], bf16, isOutput=True)

    w1_ap = w1_d.ap()  # [e, chunk, 128, KT1, w1_chunk]
    w2_ap = w2_d.ap()  # [e, fblock, 128, w2_block, D]
    w1q_ap, w1h_ap = w1q_d.ap(), w1h_d.ap()
    w2q_ap, w2h_ap, w2x_ap = w2q_d.ap(), w2h_d.ap(), w2x_d.ap()

    with tile.TileContext(nc) as tc:
        with (
            tc.tile_pool(name="const", bufs=1) as const_pool,
            tc.tile_pool(name="w1", bufs=cfg["w1_bufs"]) as w1_pool,
            tc.tile_pool(name="w2", bufs=cfg["w2_bufs"]) as w2_pool,
            tc.tile_pool(name="hs", bufs=cfg["hs_bufs"]) as hs_pool,
            tc.tile_pool(name="ht", bufs=cfg["ht_bufs"]) as ht_pool,
            tc.tile_pool(name="os", bufs=cfg["os_bufs"]) as os_pool,
            tc.tile_pool(name="w1f", bufs=cfg["w1f_bufs"]) as w1f_pool,
            tc.tile_pool(name="w2f", bufs=cfg["w2f_bufs"]) as w2f_pool,
            tc.tile_pool(name="w1h", bufs=cfg["w1f_bufs"]) as w1h_pool,
            tc.tile_pool(name="w2h", bufs=cfg["w2f_bufs"]) as w2h_pool,
            tc.tile_pool(name="w2x", bufs=cfg["w2f_bufs"]) as w2x_pool,
            tc.tile_pool(name="ph", bufs=cfg["ph_bufs"], space="PSUM") as ph_pool,
            tc.tile_pool(name="pt", bufs=cfg["pt_bufs"], space="PSUM") as pt_pool,
            tc.tile_pool(name="po", bufs=cfg["po_bufs"], space="PSUM") as po_pool,
        ):
            pools = dict(
                w1=w1_pool, w2=w2_pool, hs=hs_pool, ht=ht_pool, os=os_pool,
                w1f=w1f_pool, w2f=w2f_pool, w1h=w1h_pool, w2h=w2h_pool, w2x=w2x_pool,
                ph=ph_pool, pt=pt_pool, po=po_pool,
            )
            # Consts go on the Pool queue: cheap dispatch there, and it keeps
            # the SP queue free so the first w1 DMA starts ~2us earlier.
            xT_sb = const_pool.tile([P, E_LOC, KT1, C], bf16, tag="xt")
            nc.gpsimd.dma_start(out=xT_sb, in_=xT_d.ap())
            b1_sb = const_pool.tile([P, E_LOC, FT], f32, tag="b1")
            nc.gpsimd.dma_start(out=b1_sb, in_=b1_d.ap())
            b2_sb = const_pool.tile([1, E_LOC, D], bf16, tag="b2")
            nc.gpsimd.dma_start(out=b2_sb, in_=b2_d.ap())
            on_sb = const_pool.tile([1, C], bf16, tag="on")
            nc.gpsimd.dma_start(out=on_sb, in_=on_d.ap())
            id_sb = const_pool.tile([C, C], bf16, tag="id")
            nc.gpsimd.dma_start(out=id_sb, in_=id_d.ap())

            consts = (xT_sb, b1_sb, b2_sb, on_sb, id_sb)

            # repeats>1 wraps the computation in a hardware loop so a single
            # execute measures R back-to-back runs (benchmarking only).
            rep_ctx = (
                tc.For_i(0, repeats, 1) if repeats > 1 else contextlib.nullcontext()
            )
            with rep_ctx:
                _emit_body(
                    nc, GELU, COPY, consts, (w1_ap, w1q_ap, w1h_ap),
                    (w2_ap, w2q_ap, w2h_ap, w2x_ap), out_d, pools, f32, (bf16, fp8), cfg,
                )

    nc.compile()
    return nc


def _emit_body(nc, GELU, COPY, consts, w1_aps, w2_aps, out_d, pools, f32, dts, cfg):
    xT_sb, b1_sb, b2_sb, on_sb, id_sb = consts
    w1_ap, w1q_ap, w1h_ap = w1_aps
    w2_ap, w2q_ap, w2h_ap, w2x_ap = w2_aps
    bf16, fp8 = dts
    q8 = cfg["fp8_ftiles"]
    w1_chunk = cfg["w1_chunk"]
    n_chunks = F // w1_chunk
    tpc = w1_chunk // P  # f-tiles per chunk (transposes / mm2 steps)
    LAG = cfg["mm2_lag"]  # consume step g-LAG while mm1 runs step g
    LEAD = cfg["w1_lead"]  # w1 DMA emission runs LEAD steps ahead of w2's
    assert w1_chunk == NMAX, "hp PSUM tile is one 512-fp32 bank"
    assert cfg["w2_block"] == tpc, "w2 DMA granularity must match chunk f-tiles"
    assert cfg["w1_bufs"] >= LEAD + 2 and cfg["w2_bufs"] >= LAG + 2
    assert cfg["hs_bufs"] >= LAG + 2 and cfg["po_bufs"] >= (4 if LAG >= 1 else 2)
    assert cfg["w1f_bufs"] >= 2 and cfg["w2f_bufs"] >= 2

    # Flat software pipeline over global chunk index g. Three goals:
    #  - mm2(g) emitted LAG chunks behind mm1(g): PE never waits on the
    #    transpose->gelu latency chain, and at the kernel tail the last w2
    #    bytes feed an mm2 whose hT inputs are already computed.
    #  - BOTH weight streams issue from the SP queue, strictly alternating
    #    w1(g+LEAD), w2(g): the DMA engines serve transfers in arrival order,
    #    so a single-queue emission pins the on-wire order. w1 (which feeds
    #    the deep mm1->gelu->mm2 chain) stays exactly LEAD transfers ahead of
    #    w2 (whose mm2 consumer is immediately ready), instead of one pool
    #    racing arbitrarily far ahead and bunching the other at the tail.
    #    SP carries no compute, so its head-of-line buffer waits cannot
    #    deadlock against the ACT gelus that retire w2 buffers.
    # Per-expert step table: the fp8 f-tiles and the bf16 remainder of former
    # chunk 0 are separate pipeline steps (each step owns a full PSUM bank, so
    # accumulation groups never share a zero region). steps[i] =
    # (kind, w1 src ap, w2 src ap, n f-tiles, ft base).
    def steps_of(e):
        out = [
            ("q", w1q_ap[e], w2q_ap[e], q8, 0),
            ("h", w1h_ap[e], (w2x_ap[e], w2h_ap[e]), tpc - q8, q8),
        ]
        for i in range(n_chunks - 1):
            out.append(("b", w1_ap[e, i], w2_ap[e, i], tpc, (i + 1) * tpc))
        if e == E_LOC - 1:
            # The last expert consumes its small fp8 mini-step LAST: the mm2
            # work exposed after the final weight byte lands shrinks from a
            # full chunk (8 matmuls) to the mini (4), shortening the kernel
            # tail. (stop= is keyed on consumption count, so order is free.)
            out = out[1:] + out[:1]
        return out

    SPE = n_chunks + 1  # steps per expert
    G = E_LOC * SPE
    state = {}  # e -> (hT, op0, op1)
    w1q, w2q, hsq = {}, {}, {}

    W1_POOL = dict(q=("w1f", fp8), h=("w1h", bf16), b=("w1", bf16))
    W2_POOL = dict(q=("w2f", fp8), h=("w2h", bf16), b=("w2", bf16))

    def issue_w1(g):
        if g >= G:
            return
        e, s = divmod(g, SPE)
        kind, w1src, _, nt, _ = steps_of(e)[s]
        pool, dt = W1_POOL[kind]
        t = pools[pool].tile([P, KT1, nt * P], dt, tag=pool)
        nc.sync.dma_start(out=t, in_=w1src)
        w1q[g] = t

    def consume(g):
        if g < 0:
            return
        e, s = divmod(g, SPE)
        _, _, _, nt, ft0 = steps_of(e)[s]
        hT, op0, op1, done = state[e]
        hs = hsq.pop(g)
        w2t = w2q.pop(g)
        for t in range(nt):
            if isinstance(w2t, tuple):
                w2src_t, tt = (w2t[0], 0) if t == 0 else (w2t[1], t - 1)
            else:
                w2src_t, tt = w2t, t
            ft = ft0 + t
            done[0] += 1
            last = done[0] == FT
            tp = pools["pt"].tile([P, C], bf16, tag="tp")
            nc.tensor.transpose(tp, in_=hs[:, t * P : (t + 1) * P], identity=id_sb)
            nc.scalar.activation(
                out=hT[:, ft, :], in_=tp, func=GELU, bias=b1_sb[:, e, ft : ft + 1]
            )
            nc.tensor.matmul(
                op0,
                lhsT=hT[:, ft, :],
                rhs=w2src_t[:, tt, 0:NMAX],
                start=False,
                stop=last,
            )
            nc.tensor.matmul(
                op1,
                lhsT=hT[:, ft, :],
                rhs=w2src_t[:, tt, NMAX:D],
                start=False,
                stop=last,
            )
        if s == SPE - 1:
            # Evacuation and the out store stay OFF the ACT queue mid-stream:
            # any extra work there delays gelus, which the in-order PE turns
            # into mm2 stalls and ultimately an expert-boundary DMA gap. DVE
            # has slack for both copies; Pool (SWDGE) carries the out stores.
            # The LAST expert has no gelus left to protect, so there the two
            # halves drain in parallel (DVE + ACT) and the out store takes the
            # lower-latency ACT HWDGE path — it is the kernel's critical tail.
            os_t = pools["os"].tile([C, D], bf16, tag="os")
            nc.vector.tensor_copy(out=os_t[:, 0:NMAX], in_=op0)
            if e == E_LOC - 1:
                nc.scalar.activation(out=os_t[:, NMAX:D], in_=op1, func=COPY)
                nc.scalar.dma_start(out=out_d.ap()[e * C : (e + 1) * C, :], in_=os_t)
            else:
                nc.vector.tensor_copy(out=os_t[:, NMAX:D], in_=op1)
                nc.gpsimd.dma_start(out=out_d.ap()[e * C : (e + 1) * C, :], in_=os_t)
            del state[e]

    for g in range(LEAD):
        issue_w1(g)
    for g in range(G):
        e, s = divmod(g, SPE)
        kind, _, w2src, nt, _ = steps_of(e)[s]
        if s == 0:
            hT = pools["ht"].tile([P, FT, C], bf16, tag="ht")
            # b2 enters PSUM via a rank-1 accumulate (ones[1,C].T @ b2[1,D]) so
            # the bias never needs a C-row broadcast in HBM/SBUF.
            op0 = pools["po"].tile([C, NMAX], f32, tag="op")
            op1 = pools["po"].tile([C, NMAX], f32, tag="op")
            nc.tensor.matmul(
                op0, lhsT=on_sb, rhs=b2_sb[:, e, 0:NMAX], start=True, stop=False
            )
            nc.tensor.matmul(
                op1, lhsT=on_sb, rhs=b2_sb[:, e, NMAX:D], start=True, stop=False
            )
            state[e] = (hT, op0, op1, [0])
        consume(g - LAG)
        issue_w1(g + LEAD)
        if kind == "h":
            xsrc, hsrc = w2src
            tx = pools["w2x"].tile([P, 1, D], fp8, tag="w2x")
            nc.sync.dma_start(out=tx, in_=xsrc)
            th = pools["w2h"].tile([P, nt - 1, D], bf16, tag="w2h")
            nc.sync.dma_start(out=th, in_=hsrc)
            w2q[g] = (tx, th)
        else:
            pool, dt = W2_POOL[kind]
            w2t = pools[pool].tile([P, nt, D], dt, tag=pool)
            nc.sync.dma_start(out=w2t, in_=w2src)
            w2q[g] = w2t
        w1t = w1q.pop(g)
        hp = pools["ph"].tile([C, w1_chunk], f32, tag="hp")
        for k in range(KT1):
            nc.tensor.matmul(
                hp[:, 0 : nt * P],
                lhsT=xT_sb[:, e, k, :],
                rhs=w1t[:, k, :],
                start=(k == 0),
                stop=(k == KT1 - 1),
            )
        hs = pools["hs"].tile([C, w1_chunk], bf16, tag="hs")
        nc.vector.tensor_copy(out=hs[:, 0 : nt * P], in_=hp[:, 0 : nt * P])
        hsq[g] = hs
    for g in range(G - LAG, G):
        consume(g)


def _get_program(act="gelu", repeats=1, cfg=None):
    key = (act, repeats, tuple(sorted((cfg or {}).items())))
    if key not in _CACHE:
        _CACHE[key] = _build_program(act, repeats, cfg)
    return _CACHE[key]


def make_in_maps(x, W1, b1, W2, b2):
    import ml_dtypes

    bf16 = ml_dtypes.bfloat16
    fp8 = ml_dtypes.float8_e4m3
    chunk = DEFAULT_CFG["w1_chunk"]
    wb = DEFAULT_CFG["w2_block"]
    q8 = DEFAULT_CFG["fp8_ftiles"]
    x = np.ascontiguousarray(np.asarray(x, dtype=np.float32))
    W1 = np.asarray(W1, dtype=np.float32)
    b1 = np.ascontiguousarray(np.asarray(b1, dtype=np.float32))
    W2 = np.asarray(W2, dtype=np.float32)
    b2 = np.ascontiguousarray(np.asarray(b2, dtype=np.float32))
    ident = np.eye(C, dtype=bf16)
    in_maps = []
    for i in range(N_CORES):
        lo, hi = i * E_LOC, (i + 1) * E_LOC
        xc = x[0, lo * C : hi * C, :].reshape(E_LOC, C, KT1, P)
        xT = np.ascontiguousarray(xc.transpose(3, 0, 2, 1)).astype(bf16)  # [128,e,k,c]
        b1t = np.ascontiguousarray(
            b1[lo:hi].reshape(E_LOC, FT, P).transpose(2, 0, 1)
        )  # [128, e, ft]
        b2s = np.ascontiguousarray(b2[lo:hi][None]).astype(bf16)  # [1, e, d]
        # [e, chunkidx, p, k, fc]; chunk 0 splits into an fp8 piece (first q8
        # f-tiles) and a bf16 remainder; chunks 1.. stay whole in bf16.
        n_ch = F // chunk
        w1full = W1[lo:hi].reshape(E_LOC, KT1, P, n_ch, chunk).transpose(0, 3, 2, 1, 4)
        w2full = W2[lo:hi].reshape(E_LOC, FT // wb, wb, P, D).transpose(0, 1, 3, 2, 4)
        m = {
            "xT": xT,
            "w1": np.ascontiguousarray(w1full[:, 1:]).astype(bf16),
            "w2": np.ascontiguousarray(w2full[:, 1:]).astype(bf16),
            "w1q": np.ascontiguousarray(w1full[:, 0, :, :, : q8 * P]).astype(fp8),
            "w1h": np.ascontiguousarray(w1full[:, 0, :, :, q8 * P :]).astype(bf16),
            "w2q": np.ascontiguousarray(w2full[:, 0, :, :q8]).astype(fp8),
            "w2x": np.ascontiguousarray(w2full[:, 0, :, q8 : q8 + 1]).astype(fp8),
            "w2h": np.ascontiguousarray(w2full[:, 0, :, q8 + 1 :]).astype(bf16),
            "b1t": b1t,
            "b2s": b2s,
            "ones1": np.ones((1, C), dtype=bf16),
            "ident": ident,
        }
        in_maps.append(m)
    return in_maps


def kernel(x, W1, b1, W2, b2):
    global LAST_RESULTS
    from concourse.bass_utils import run_bass_kernel_spmd

    nc = _get_program()
    in_maps = make_in_maps(x, W1, b1, W2, b2)
    trace = bool(int(os.environ.get("KERNEL_TRACE", "0")))
    res = run_bass_kernel_spmd(nc, in_maps, list(range(N_CORES)), trace=trace)
    LAST_RESULTS = res
    out = np.concatenate([np.asarray(r["out"]) for r in res.results], axis=0)
    return out.reshape(1, E * C, D).astype(np.float32)



# revision 16
# speedup vs baseline: 1.6765x; 1.6765x over previous
"""MoE grouped-FFN kernel for Trainium2 (8 NeuronCores, expert-parallel).

Problem: x [1, 2048, 1024] fp32, 32 experts x 64 tokens each,
per-expert FFN 1024 -> 4096 (gelu) -> 1024.

Sharding: expert-parallel, 4 experts per core. Tokens are statically
pre-chunked per expert (dim 1 == E*C), so each core just gets its 4
experts' token rows + weights; outputs concatenate back. No collectives.

The problem is HBM-bound, so weights stream in fp8 e3m4 (1 byte/elem,
4 mantissa bits — 2x the precision of e4m3 at the same width). All
weight chunks are pre-scaled by SCALE=64 on the host so values land in
e3m4's normal range (sigma*64 ~ 1.28 vs e3m4 normals [0.25, 15.5]);
bf16 fallback chunks (n_q1/n_q2 knobs) get the same exact power-of-2
scale so every PSUM accumulation is uniformly 64-scaled regardless of
source dtype. Unscales ride existing ops: gelu's activation computes
func(in*scale + bias) with scale=1/64, and the mm2 evacuation Copy
carries scale=1/64. Activations (xT, h, out) are fp16 — same bytes as
bf16, 8x less rounding noise. Measured on HW: rel err 1.77e-2 vs the
2e-2 gate (numpy sim predicted 1.764e-2).

Dataflow is WEIGHT-STATIONARY both matmuls: the [128,128] weight tile
is the PE stationary operand and the 64 tokens stream as the moving
operand. A fresh fp8 stationary per 64-column matmul sustains ~45ns
(measured: LD_WEIGHTS mostly overlaps), so PE busy is ~93us/core vs
121us for the token-stationary form — and mm1's PSUM output lands
[f, c], which kills the PE transposes and DVE copies the
token-stationary form needed (gelu reads PSUM directly with b1 as a
per-partition bias), and mm2's PSUM lands [d, c] where b2 is a cheap
per-d rank-1 and the host un-transposes the stored output for free.

Per-core dataflow, software-pipelined over 8 uniform 512-wide chunks
per expert, both weight streams issued from the SP queue strictly
alternating w1(g+LEAD), w2(g) (single-queue emission pins the on-wire
DMA order; w1 feeds the deeper mm1->gelu->mm2 chain):
  mm1: h[128f, c] += W1[k,ftile].T @ xT[k]      (8 k-tiles chained per f-tile)
  ACT: hT[ft] = gelu(h_psum/64 + b1[f])         (PSUM read, per-partition bias)
  mm2: oT[128d, c] += W2[ft,dblk].T @ hT[ft]    (32 f-tiles chained per d-block;
       b2*64 enters mid-group as a rank-1 b2[1,128d].T @ ones[1,c])
  ACT evacuates oT*(1/64) to fp16 [128d, 8, c]; host un-transposes.
"""

import os
import numpy as np

E, C, D, F = 32, 64, 1024, 4096
N_CORES = 8
E_LOC = E // N_CORES  # experts per core
P = 128
KT1 = D // P  # 8 K-tiles in mm1; also 8 d-blocks in mm2's output
FT = F // P  # 32 f-tiles
SCALE = 64.0  # host pre-scale on all weight chunks (power of 2: exact in bf16)

DEFAULT_CFG = dict(
    n_q1=8,        # leading W1 chunks (of 8) streamed in e3m4; rest bf16
    n_q2=8,        # same for W2
    w1_bufs=6,
    w2_bufs=8,
    ht_bufs=2,
    os_bufs=2,
    ph_bufs=3,
    po_bufs=2,
    mm2_lag=2,     # consume chunk g-LAG behind mm1 emission
    w1_lead=3,     # w1 transfer stream leads w2 by this many chunks
)

_CACHE = {}
LAST_RESULTS = None  # BassKernelResults of the most recent run (for profiling)
TPC = 4  # f-tiles per 512-wide chunk
N_CH = FT // TPC  # 8 chunks per expert


def _build_program(act="gelu", repeats=1, cfg=None):
    import contextlib

    import concourse.bacc as bacc
    import concourse.tile as tile
    import concourse.mybir as mybir

    cfg = dict(DEFAULT_CFG, **(cfg or {}))

    f32 = mybir.dt.float32
    fp16 = mybir.dt.float16
    bf16 = mybir.dt.bfloat16
    fp8 = mybir.dt.float8e3  # e3m4
    # CoreSim doesn't implement the Gelu LUTs; "tanh" is a sim-only stand-in
    # used by test.py to validate everything except the activation itself.
    GELU = {
        "gelu": mybir.ActivationFunctionType.Gelu_apprx_tanh,
        "tanh": mybir.ActivationFunctionType.Tanh,
    }[act]
    COPY = mybir.ActivationFunctionType.Copy

    nc = bacc.Bacc("TRN2", target_bir_lowering=False, debug=False)

    nq1, nq2 = cfg["n_q1"], cfg["n_q2"]
    assert 0 <= nq1 <= N_CH and 0 <= nq2 <= N_CH
    xT_d = nc.declare_dram_parameter("xT", [P, E_LOC, KT1, C], fp16, isOutput=False)
    # Weights arrive host-pre-tiled (and pre-scaled by SCALE) so every weight
    # DMA is one contiguous read of 4KB per partition:
    # w1[e, c, p, t, k, fc] = SCALE*W1[e, k*128+p, (c*4+t)*128+fc]
    # w2[e, c, p, t, j, dc] = SCALE*W2[e, (c*4+t)*128+p, j*128+dc]
    # The first nq chunks live in the e3m4 params, the rest in bf16 params.
    w_aps = {}
    for nm, nq in (("w1", nq1), ("w2", nq2)):
        q = h = None
        shp = [P, TPC, KT1, P]
        if nq > 0:
            q = nc.declare_dram_parameter(
                nm + "q", [E_LOC, nq] + shp, fp8, isOutput=False
            ).ap()
        if nq < N_CH:
            h = nc.declare_dram_parameter(
                nm + "h", [E_LOC, N_CH - nq] + shp, bf16, isOutput=False
            ).ap()
        w_aps[nm] = (q, h, nq)
    b1_d = nc.declare_dram_parameter("b1t", [P, E_LOC, FT], f32, isOutput=False)
    b2_d = nc.declare_dram_parameter("b2s", [1, E_LOC, D], fp16, isOutput=False)
    on_d = nc.declare_dram_parameter("ones1", [1, C], fp16, isOutput=False)
    out_d = nc.declare_dram_parameter("out", [E_LOC, P, KT1, C], fp16, isOutput=True)

    with tile.TileContext(nc) as tc:
        with (
            tc.tile_pool(name="const", bufs=1) as const_pool,
            tc.tile_pool(name="w1", bufs=cfg["w1_bufs"]) as w1_pool,
            tc.tile_pool(name="w2", bufs=cfg["w2_bufs"]) as w2_pool,
            tc.tile_pool(name="ht", bufs=cfg["ht_bufs"]) as ht_pool,
            tc.tile_pool(name="os", bufs=cfg["os_bufs"]) as os_pool,
            tc.tile_pool(name="ph", bufs=cfg["ph_bufs"], space="PSUM") as ph_pool,
            tc.tile_pool(name="po", bufs=cfg["po_bufs"], space="PSUM") as po_pool,
        ):
            pools = dict(
                w1=w1_pool, w2=w2_pool, ht=ht_pool, os=os_pool,
                ph=ph_pool, po=po_pool,
            )
            # Consts ride the Pool/SWDGE queue so the SP queue's first w1
            # DMA is never delayed. Only xT[e0] gates the first matmul; b1
            # must land by the first gelu (~5.5us); b2s/ones by the first
            # mid-expert bias rank-1 (~15us); xT[e>0] by expert e (~25us+).
            xT_sb = const_pool.tile([P, E_LOC, KT1, C], fp16, tag="xt")
            nc.gpsimd.dma_start(out=xT_sb[:, 0], in_=xT_d.ap()[:, 0])

            def late_consts():
                b1_sb = const_pool.tile([P, E_LOC, FT], f32, tag="b1")
                nc.gpsimd.dma_start(out=b1_sb, in_=b1_d.ap())
                b2_sb = const_pool.tile([1, E_LOC, D], fp16, tag="b2")
                nc.gpsimd.dma_start(out=b2_sb, in_=b2_d.ap())
                on_sb = const_pool.tile([1, C], fp16, tag="on")
                nc.gpsimd.dma_start(out=on_sb, in_=on_d.ap())
                for e in range(1, E_LOC):
                    nc.gpsimd.dma_start(out=xT_sb[:, e], in_=xT_d.ap()[:, e])
                return b1_sb, b2_sb, on_sb

            consts = (xT_sb, late_consts)

            # repeats>1 wraps the computation in a hardware loop so a single
            # execute measures R back-to-back runs (benchmarking only).
            rep_ctx = (
                tc.For_i(0, repeats, 1) if repeats > 1 else contextlib.nullcontext()
            )
            with rep_ctx:
                _emit_body(
                    nc, GELU, COPY, consts, w_aps, out_d, pools,
                    (f32, fp16, bf16, fp8), cfg,
                )

    nc.compile()
    return nc


def _emit_body(nc, GELU, COPY, consts, w_aps, out_d, pools, dts, cfg):
    xT_sb, late_consts = consts
    f32, fp16, bf16, fp8 = dts
    LAG = cfg["mm2_lag"]  # consume (mm2) chunk g-LAG while mm1 runs chunk g
    LEAD = cfg["w1_lead"]  # w1 DMA emission runs LEAD steps ahead of w2's
    INV = 1.0 / SCALE
    assert cfg["w1_bufs"] >= LEAD + 2 and cfg["w2_bufs"] >= LAG + 2
    assert cfg["ph_bufs"] >= LAG + 1 and cfg["po_bufs"] >= 2

    def src_of(which, e, s):
        q, h, nq = w_aps[which]
        return (q[e, s], fp8) if s < nq else (h[e, s - nq], bf16)

    SPE = N_CH  # pipeline steps per expert
    G = E_LOC * SPE
    state = {}  # e -> (hT, oT, done)
    w1q, w2q, phq = {}, {}, {}
    lc = [None]

    def issue_w1(g):
        if g >= G:
            return
        e, s = divmod(g, SPE)
        src, dt = src_of("w1", e, s)
        t = pools["w1"].tile([P, TPC, KT1, P], dt, tag="w1" + dt.name)
        nc.sync.dma_start(out=t, in_=src)
        w1q[g] = t

    def issue_w2(g):
        e, s = divmod(g, SPE)
        src, dt = src_of("w2", e, s)
        if e == E_LOC - 1 and s == SPE - 1:
            # Tail split 3+1: the final mm2's weight DMA shrinks to one
            # f-tile, so the work exposed after the last weight byte lands
            # is 8 short matmuls instead of 32.
            ta = pools["w2"].tile([P, TPC - 1, KT1, P], dt, tag="w2" + dt.name)
            nc.sync.dma_start(out=ta, in_=src[:, 0 : TPC - 1])
            tb = pools["w2"].tile([P, 1, KT1, P], dt, tag="w2b" + dt.name)
            nc.sync.dma_start(out=tb, in_=src[:, TPC - 1 : TPC])
            w2q[g] = (ta, tb)
        else:
            t = pools["w2"].tile([P, TPC, KT1, P], dt, tag="w2" + dt.name)
            nc.sync.dma_start(out=t, in_=src)
            w2q[g] = t

    def mm1(g):
        e, s = divmod(g, SPE)
        w1t = w1q.pop(g)
        hp = pools["ph"].tile([P, TPC, C], f32, tag="hp")
        b1_sb = lc[0][0]
        # One accumulation group per chunk: start=True zeroes the WHOLE 2KB
        # PSUM bank (the hardware zero region), so the 4 f-tiles of a chunk
        # must share a single group — first matmul starts it, last stops it,
        # every matmul accumulates its own [128, 64] slice of the bank.
        # k-major order interleaves the 4 f-tile slices (measured ~42 vs
        # 45.5 ns/matmul for straight chains) and reuses one xT[k] moving
        # operand for 4 consecutive matmuls.
        for k in range(KT1):
            for t in range(TPC):
                nc.tensor.matmul(
                    hp[:, t, :],
                    lhsT=w1t[:, t, k, :],
                    rhs=xT_sb[:, e, k, :],
                    start=(k == 0 and t == 0),
                    stop=(k == KT1 - 1 and t == TPC - 1),
                )
        for t in range(TPC):
            ft = s * TPC + t
            # gelu reads the PSUM f-tile directly: per-partition bias b1[f],
            # scale folds the 1/64 weight unquant. ACT runs these while PE
            # moves on to the next chunk.
            nc.scalar.activation(
                out=state[e][0][:, ft, :], in_=hp[:, t, :], func=GELU,
                bias=b1_sb[:, e, ft : ft + 1], scale=INV,
            )
        phq[g] = hp

    def consume_m(g):
        if g < 0:
            return
        e, s = divmod(g, SPE)
        hT, oT, done = state[e]
        b2_sb, on_sb = lc[0][1], lc[0][2]
        w2t = w2q.pop(g)
        for t in range(TPC):
            if isinstance(w2t, tuple):
                wt, tt = (w2t[0], t) if t < TPC - 1 else (w2t[1], 0)
            else:
                wt, tt = w2t, t
            ft = s * TPC + t
            done[0] += 1
            first = done[0] == 1
            last = done[0] == FT
            # Like mm1, the 8 d-block slots share the expert's single PSUM
            # bank and therefore a single accumulation group: only the very
            # first matmul of the expert starts it, only the very last stops.
            for j in range(KT1):
                nc.tensor.matmul(
                    oT[:, j, :],
                    lhsT=wt[:, tt, j, :],
                    rhs=hT[:, ft, :],
                    start=(first and j == 0),
                    stop=(last and j == KT1 - 1),
                )
            if done[0] == FT // 2:
                # b2 enters PSUM as a rank-1 accumulate per d-block:
                # b2s[1,128d].T @ ones[1,C]. Mid-group emission (legal
                # anywhere between start and stop) keeps it off both the
                # kernel warmup and the expert tail. b2s is host-scaled by
                # SCALE to match the 64-scaled accumulation.
                for j in range(KT1):
                    nc.tensor.matmul(
                        oT[:, j, :],
                        lhsT=b2_sb[:, e, j * P : (j + 1) * P],
                        rhs=on_sb,
                        start=False,
                        stop=False,
                    )
        if s == SPE - 1:
            # Evacuation: one ACT Copy (scale=1/64) [128, 512] per expert;
            # the out store rides Pool/SWDGE mid-stream. The LAST expert
            # splits halves across DVE + ACT with stores on separate DGE
            # units (Pool SWDGE + ACT HWDGE) so the two tails drain in
            # parallel — it is the kernel's critical tail.
            os_t = pools["os"].tile([P, KT1, C], fp16, tag="os")
            orow = out_d.ap()[e]
            if e == E_LOC - 1:
                half = KT1 // 2
                nc.vector.tensor_scalar_mul(
                    out=os_t[:, 0:half, :], in0=oT[:, 0:half, :], scalar1=INV
                )
                nc.gpsimd.dma_start(out=orow[:, 0:half], in_=os_t[:, 0:half, :])
                nc.scalar.activation(
                    out=os_t[:, half:KT1, :], in_=oT[:, half:KT1, :],
                    func=COPY, scale=INV,
                )
                nc.scalar.dma_start(out=orow[:, half:KT1], in_=os_t[:, half:KT1, :])
            else:
                # DVE (otherwise idle in this dataflow) carries mid-stream
                # evacuations so ACT stays dedicated to gelus.
                nc.vector.tensor_scalar_mul(out=os_t, in0=oT, scalar1=INV)
                nc.gpsimd.dma_start(out=orow, in_=os_t)
            del state[e]

    for g in range(LEAD):
        issue_w1(g)
    lc[0] = late_consts()
    for g in range(G):
        e, s = divmod(g, SPE)
        if s == 0:
            hT = pools["ht"].tile([P, FT, C], fp16, tag="ht")
            oT = pools["po"].tile([P, KT1, C], f32, tag="ot")
            state[e] = (hT, oT, [0])
        issue_w1(g + LEAD)
        issue_w2(g)
        mm1(g)
        consume_m(g - LAG)
        if g - LAG - 1 >= 0:
            phq.pop(g - LAG - 1, None)
    for g in range(G - LAG, G):
        consume_m(g)


def _get_program(act="gelu", repeats=1, cfg=None):
    key = (act, repeats, tuple(sorted((cfg or {}).items())))
    if key not in _CACHE:
        _CACHE[key] = _build_program(act, repeats, cfg)
    return _CACHE[key]


def make_in_maps(x, W1, b1, W2, b2, cfg=None):
    import ml_dtypes

    bf16 = ml_dtypes.bfloat16
    fp8 = ml_dtypes.float8_e3m4
    fp16 = np.float16
    cfg = dict(DEFAULT_CFG, **(cfg or {}))
    nq1, nq2 = cfg["n_q1"], cfg["n_q2"]
    x = np.ascontiguousarray(np.asarray(x, dtype=np.float32))
    W1 = np.asarray(W1, dtype=np.float32)
    b1 = np.ascontiguousarray(np.asarray(b1, dtype=np.float32))
    W2 = np.asarray(W2, dtype=np.float32)
    b2 = np.ascontiguousarray(np.asarray(b2, dtype=np.float32))
    in_maps = []
    for i in range(N_CORES):
        lo, hi = i * E_LOC, (i + 1) * E_LOC
        xc = x[0, lo * C : hi * C, :].reshape(E_LOC, C, KT1, P)
        xT = np.ascontiguousarray(xc.transpose(3, 0, 2, 1)).astype(fp16)  # [128,e,k,c]
        b1t = np.ascontiguousarray(
            b1[lo:hi].reshape(E_LOC, FT, P).transpose(2, 0, 1)
        )  # [128, e, ft]
        b2s = np.ascontiguousarray(b2[lo:hi][None] * SCALE).astype(fp16)  # [1, e, d]
        # [e, chunk, p, t, k/j, 128], pre-scaled by SCALE (exact in bf16 too);
        # first nq chunks e3m4, rest bf16.
        w1full = (W1[lo:hi] * SCALE).reshape(E_LOC, KT1, P, N_CH, TPC, P)
        w1full = w1full.transpose(0, 3, 2, 4, 1, 5)
        w2full = (W2[lo:hi] * SCALE).reshape(E_LOC, N_CH, TPC, P, KT1, P)
        w2full = w2full.transpose(0, 1, 3, 2, 4, 5)
        m = {
            "xT": xT,
            "b1t": b1t,
            "b2s": b2s,
            "ones1": np.ones((1, C), dtype=fp16),
        }
        if nq1 > 0:
            m["w1q"] = np.ascontiguousarray(w1full[:, :nq1]).astype(fp8)
        if nq1 < N_CH:
            m["w1h"] = np.ascontiguousarray(w1full[:, nq1:]).astype(bf16)
        if nq2 > 0:
            m["w2q"] = np.ascontiguousarray(w2full[:, :nq2]).astype(fp8)
        if nq2 < N_CH:
            m["w2h"] = np.ascontiguousarray(w2full[:, nq2:]).astype(bf16)
        in_maps.append(m)
    return in_maps


def unshuffle_out(out_t):
    """[E_LOC, 128p, 8j, 64c] (transposed d-major device layout) ->
    [E_LOC*C, D] with out[e*64+c, j*128+p]."""
    return np.ascontiguousarray(
        np.asarray(out_t).transpose(0, 3, 2, 1)
    ).reshape(E_LOC * C, D)


def kernel(x, W1, b1, W2, b2):
    global LAST_RESULTS
    from concourse.bass_utils import run_bass_kernel_spmd

    nc = _get_program()
    in_maps = make_in_maps(x, W1, b1, W2, b2)
    trace = bool(int(os.environ.get("KERNEL_TRACE", "0")))
    res = run_bass_kernel_spmd(nc, in_maps, list(range(N_CORES)), trace=trace)
    LAST_RESULTS = res
    out = np.concatenate([unshuffle_out(r["out"]) for r in res.results], axis=0)
    return out.reshape(1, E * C, D).astype(np.float32)
